# revision 2
# baseline (speedup 1.0000x reference)
"""Trainium2 Bass kernel for DigitConvolutionalModel.

Math: logits = relu(conv2d_valid(x.reshape(B,28,28), conv_w).reshape(B,676) @ W1 + b1) @ W2 + b2

Optimizations:
  1. The valid 3x3 conv is linear in x, so it folds into W1 on host:
     feat @ W1 == x @ (C @ W1) where C[784,676] scatters conv_w taps.
     The device then runs two dense matmuls per batch shard:
       h = relu(x @ W1eff + b1);  logits = h @ W2 + b2
  2. Sharding: batch 32768 split as 8 x 4096 across cores. Each shard is
     host-packed partition-major ([112, nblk, 7, 512] bf16) so a multi-
     block DMA is one fully contiguous >=14KB-per-partition transfer
     that runs near HBM line rate, and the contraction dim lands on SBUF
     partitions with no on-device transposes.
  3. x, W1eff, h, W2 are bf16 on device (fp32 PSUM accumulation): halves
     DMA bytes AND streams the PE at 1 cycle/row. Measured end-to-end
     rel err ~3.6e-3 against the fp32 reference.
  4. Startup critical path: small weights + w1 chunk 0 go first on the
     scalar HWDGE ring while x block-0 chunk groups stream on the sync
     ring, so the first real MM1 starts ~1us after the framework
     preamble instead of waiting for a 400KB weight blob. Two dummy
     matmuls on a zeroed tile start the HAM clock-ramp window early.
  5. Steady state: blocks 1..7 arrive as four ~1.6MB contiguous DMAs on
     the sync ring (nothing else shares it), keeping the PE fed at the
     ~270GB/s it consumes. The scalar ring only does relu ACTIVATEs
     after startup; logits drain per block on the idle gpsimd (SWDGE)
     ring, except the last block which drains on scalar (HWDGE latency).

Device kernel (per core, per 512-column block):
  - MM1: hT[256,512] = W1eff.T @ xT accumulated over 7 k-chunks of 112
  - ACT: relu(hT + b1) PSUM->SBUF, output bf16
  - MM2 (pipelined one block behind): logitsT[10,512] over 2 chunks
  - DVE: + b2 (per-partition scalar add) PSUM->SBUF logitsT buffer
  - per-block DMA of logitsT slice; host transposes back to [B, 10]
"""
import ml_dtypes
import numpy as np

import concourse.bacc as bacc
import concourse.mybir as mybir
from concourse.tile import TileContext
from concourse.bass_utils import run_bass_kernel_spmd

B = 32768
IMG = 28
KSZ = 3
OUT_HW = IMG - KSZ + 1  # 26
FEAT = OUT_HW * OUT_HW  # 676
PIX = IMG * IMG  # 784
HID = 256
NCLS = 10
N_CORES = 8
BC = B // N_CORES  # 4096 rows per core
NBLK_COLS = 512  # batch columns per pipeline block (1 PSUM bank of fp32)
KCH = 112  # 784 = 7 * 112 contraction chunks
NKC = PIX // KCH  # 7
# wb blob layout (per partition, fp32 words): b1[2] | b2
WB_B1 = 0          # [128, 2]
WB_B2 = 2          # [128, 1] (only partitions 0..9 used)
WB_W = 3

f32 = mybir.dt.float32
bf16 = mybir.dt.bfloat16
AF = mybir.ActivationFunctionType

X_DT = bf16
W_DT = bf16
H_DT = bf16

# x block-0 chunk-group splits (sync ring), chosen so supply tracks the
# cold-clock consumption rate; later blocks as 2-block contiguous DMAs
BLK0_SPLITS = [(0, 1), (1, 3), (3, 5), (5, 7)]
XBLK_PER_DMA = 2
N_WARMUP = 2

_CACHE = {}


def _build(bc=BC):
    """Build the single-core Bass program (SPMD across 8 cores)."""
    nblk = bc // NBLK_COLS
    nc = bacc.Bacc()
    # xT is host-packed partition-major: [partition k, block, k-chunk c,
    # batch col] so any [:, b0:b1] slice is one fully contiguous DMA.
    xT = nc.declare_dram_parameter("xT", [KCH, nblk, NKC, NBLK_COLS], X_DT, isOutput=False)
    # w1 blob: host-prearranged [112, 7, 256] (chunk-major per partition)
    w1e = nc.declare_dram_parameter("w1b", [KCH, NKC, HID], W_DT, isOutput=False)
    # w2 blob: [128, 2, 10] (chunk-major per partition)
    w2 = nc.declare_dram_parameter("w2b", [128, 2, NCLS], W_DT, isOutput=False)
    # small-weights blob: [128, WB_W] fp32, see WB_* offsets
    wb = nc.declare_dram_parameter("wb", [128, WB_W], f32, isOutput=False)
    # output is logitsT [10, bc]; host transposes back
    out = nc.declare_dram_parameter("out", [NCLS, bc], f32, isOutput=True)

    with TileContext(nc) as tc:
        with (
            tc.tile_pool(name="weights", bufs=1) as wpool,
            tc.tile_pool(name="h_sb", bufs=4) as hpool,
            tc.tile_pool(name="h_ps", bufs=4, space="PSUM") as hps,
            tc.tile_pool(name="log_ps", bufs=2, space="PSUM") as logps,
        ):
            # ---- weight staging on the scalar HWDGE ring: smalls first,
            # then w1 chunk 0 (unblocks MM1), then the rest of w1 ----
            wb_sb = wpool.tile([128, WB_W], f32)
            nc.scalar.dma_start(out=wb_sb[:], in_=wb[:])
            w2_sb = wpool.tile([128, 2, NCLS], W_DT)
            nc.scalar.dma_start(out=w2_sb[:], in_=w2[:])
            w1_sb = wpool.tile([KCH, NKC, HID], W_DT)
            nc.scalar.dma_start(out=w1_sb[:, 0:1, :], in_=w1e[:, 0:1, :])
            nc.scalar.dma_start(out=w1_sb[:, 1:NKC, :], in_=w1e[:, 1:NKC, :])

            # ---- x staging on the sync HWDGE ring: block 0 in chunk
            # groups (progressive availability), rest as big contiguous
            # multi-block DMAs ----
            x_sb = wpool.tile([KCH, nblk, NKC, NBLK_COLS], X_DT)
            for c0, c1 in BLK0_SPLITS:
                nc.sync.dma_start(out=x_sb[:, 0, c0:c1, :], in_=xT[:, 0, c0:c1, :])
            b = 1
            while b < nblk:
                be = min(b + XBLK_PER_DMA, nblk)
                nc.sync.dma_start(out=x_sb[:, b:be], in_=xT[:, b:be])
                b = be

            b1_sb = wb_sb[:, WB_B1:WB_B2]
            b2_sb = wb_sb[:NCLS, WB_B2:WB_W]
            # all blocks' logitsT accumulate here; per-block drain
            log_all = wpool.tile([NCLS, bc], f32)

            # tiny warm-up: start the HAM activity window while block-0
            # DMAs land (2 matmuls on a zeroed tile)
            warm_a = wpool.tile([KCH, 128], X_DT)
            warm_b = wpool.tile([KCH, NBLK_COLS], X_DT)
            nc.vector.memset(warm_a[:], 0.0)
            nc.vector.memset(warm_b[:], 0.0)
            warm_ps = hps.tile([128, NBLK_COLS], f32, tag="h_ps")
            for _ in range(N_WARMUP):
                nc.tensor.matmul(
                    warm_ps[:], warm_a[:], warm_b[:], start=True, stop=True,
                    skip_group_check=True,
                )

            # ---- main pipeline over 512-column blocks ----
            # MM2 for block n is emitted during block n+1's MM1 so the PE
            # never waits on the relu round-trip.
            pending = None  # (hs, b0) awaiting MM2

            def emit_mm2(hs, b0, last=False):
                log_ps = logps.tile([NCLS, NBLK_COLS], f32)
                for mc in range(2):
                    nc.tensor.matmul(
                        log_ps[:],
                        w2_sb[:, mc, :],
                        hs[mc][:],
                        start=(mc == 0),
                        stop=(mc == 1),
                    )
                nc.vector.tensor_scalar_add(
                    out=log_all[:, b0 : b0 + NBLK_COLS],
                    in0=log_ps[:],
                    scalar1=b2_sb[:, 0:1],
                )
                eng = nc.scalar if last else nc.gpsimd
                eng.dma_start(
                    out=out[:, b0 : b0 + NBLK_COLS],
                    in_=log_all[:, b0 : b0 + NBLK_COLS],
                )

            for blk in range(nblk):
                b0 = blk * NBLK_COLS
                hs = []
                for mc in range(2):
                    h_ps = hps.tile([128, NBLK_COLS], f32)
                    for kc in range(NKC):
                        nc.tensor.matmul(
                            h_ps[:],
                            w1_sb[:, kc, mc * 128 : (mc + 1) * 128],
                            x_sb[:, blk, kc, :],
                            start=(kc == 0),
                            stop=(kc == NKC - 1),
                        )
                    h_sb = hpool.tile([128, NBLK_COLS], H_DT, tag="h")
                    nc.scalar.activation(
                        h_sb[:], h_ps[:], AF.Relu, bias=b1_sb[:, mc : mc + 1]
                    )
                    hs.append(h_sb)
                    if mc == 0 and pending is not None:
                        emit_mm2(*pending)
                        pending = None
                pending = (hs, b0)

            emit_mm2(*pending, last=True)

    nc.compile()
    return nc


def _fold_conv_into_w1(conv_w, W1):
    """W1eff[784, 256] such that x @ W1eff == conv(x) flattened @ W1."""
    conv_w = np.asarray(conv_w, dtype=np.float64)
    W1 = np.asarray(W1, dtype=np.float64)
    C = np.zeros((IMG, IMG, OUT_HW, OUT_HW), dtype=np.float64)
    oi = np.arange(OUT_HW)[:, None]
    oj = np.arange(OUT_HW)[None, :]
    for ki in range(KSZ):
        for kj in range(KSZ):
            C[oi + ki, oj + kj, oi, oj] = conv_w[ki, kj]
    W1eff = C.reshape(PIX, FEAT) @ W1
    return np.ascontiguousarray(W1eff, dtype=np.float32)


def _pack_weights(w1e, b1, W2, b2):
    # w1 blob [112, 7, 256]: chunk-major per partition (matches w1_sb)
    np_wdt = mybir.dt.np(W_DT)
    w1b = np.ascontiguousarray(
        w1e.reshape(NKC, KCH, HID).transpose(1, 0, 2).astype(np_wdt)
    )
    w2b = np.ascontiguousarray(
        W2.reshape(2, 128, NCLS).transpose(1, 0, 2).astype(np_wdt)
    )
    wb = np.zeros((128, WB_W), dtype=np.float32)
    wb[:, WB_B1:WB_B2] = b1.reshape(2, 128).T
    wb[:NCLS, WB_B2] = b2
    return w1b, w2b, wb


def kernel(x, conv_w, W1, b1, W2, b2, _bc=BC, _trace=False):
    x = np.asarray(x, dtype=np.float32)
    w1e = _fold_conv_into_w1(conv_w, W1)
    b1 = np.asarray(b1, dtype=np.float32)
    W2 = np.asarray(W2, dtype=np.float32)
    b2 = np.asarray(b2, dtype=np.float32)
    w1b, w2b, wb = _pack_weights(w1e, b1, W2, b2)

    n_cores = x.shape[0] // _bc
    if _bc not in _CACHE:
        _CACHE[_bc] = _build(_bc)
    nc = _CACHE[_bc]

    nblk = _bc // NBLK_COLS
    in_maps = [
        {
            # [bc, 784] -> [nblk, 512, 7, 112] -> [112, nblk, 7, 512]
            "xT": np.ascontiguousarray(
                x[c * _bc : (c + 1) * _bc]
                .reshape(nblk, NBLK_COLS, NKC, KCH)
                .transpose(3, 0, 2, 1)
                .astype(mybir.dt.np(X_DT))
            ),
            "w1b": w1b,
            "w2b": w2b,
            "wb": wb,
        }
        for c in range(n_cores)
    ]
    res = run_bass_kernel_spmd(
        nc, in_maps, core_ids=list(range(n_cores)), trace=_trace
    )
    # device layout logitsT [10, bc] -> [bc, 10]
    out = np.concatenate(
        [np.ascontiguousarray(res.results[c]["out"].T) for c in range(n_cores)],
        axis=0,
    )
    if _trace:
        return out, res
    return out


# revision 7
# speedup vs baseline: 1.1542x; 1.1542x over previous
"""Trainium2 Bass kernel for DigitConvolutionalModel.

Math: logits = relu(conv2d_valid(x.reshape(B,28,28), conv_w).reshape(B,676) @ W1 + b1) @ W2 + b2

Optimizations:
  1. The valid 3x3 conv is linear in x, so it folds into W1 on host:
     feat @ W1 == x @ (C @ W1) where C[784,676] scatters conv_w taps.
     The device then runs two dense matmuls per batch shard:
       h = relu(x @ W1eff + b1);  logits = h @ W2 + b2
  2. Sharding: batch 32768 split as 8 x 4096 across cores. Each shard is
     host-packed partition-major ([112, nblk, 7, 512] bf16) so a multi-
     block DMA is one fully contiguous >=14KB-per-partition transfer
     that runs near HBM line rate, and the contraction dim lands on SBUF
     partitions with no on-device transposes.
  3. x, W1eff, h, W2 are bf16 on device (fp32 PSUM accumulation): halves
     DMA bytes AND streams the PE at 1 cycle/row. Measured end-to-end
     rel err ~3.6e-3 against the fp32 reference.
  4. Startup critical path: small weights + w1 chunk 0 go first on the
     scalar HWDGE ring while x block-0 chunk groups stream on the sync
     ring, so the first real MM1 starts ~1us after the framework
     preamble instead of waiting for a 400KB weight blob. Two dummy
     matmuls on a zeroed tile start the HAM clock-ramp window early.
  5. Steady state: blocks 1..7 arrive as four ~1.6MB contiguous DMAs on
     the sync ring (nothing else shares it), keeping the PE fed at the
     ~270GB/s it consumes. The scalar ring only does relu ACTIVATEs
     after startup; logits drain per block on the idle gpsimd (SWDGE)
     ring, except the last block which drains on scalar (HWDGE latency).

Device kernel (per core, per 512-column block):
  - MM1: hT[256,512] = W1eff.T @ xT accumulated over 7 k-chunks of 112
  - ACT: relu(hT + b1) PSUM->SBUF, output bf16
  - MM2 (pipelined one block behind): logitsT[10,512] over 2 chunks
  - DVE: + b2 (per-partition scalar add) PSUM->SBUF logitsT buffer
  - per-block DMA of logitsT slice; host transposes back to [B, 10]
"""
import ml_dtypes
import numpy as np

import concourse.bacc as bacc
import concourse.mybir as mybir
from concourse.tile import TileContext
from concourse.bass_utils import run_bass_kernel_spmd

B = 32768
IMG = 28
KSZ = 3
OUT_HW = IMG - KSZ + 1  # 26
FEAT = OUT_HW * OUT_HW  # 676
PIX = IMG * IMG  # 784
HID = 256
NCLS = 10
N_CORES = 8
BC = B // N_CORES  # 4096 rows per core
NBLK_COLS = 512  # batch columns per pipeline block (1 PSUM bank of fp32)
KCH = 112  # 784 = 7 * 112 contraction chunks
NKC = PIX // KCH  # 7
# wb blob layout (per partition, fp32 words): b1[2] | b2
WB_B1 = 0          # [128, 2]
WB_B2 = 2          # [128, 1] (only partitions 0..9 used)
WB_W = 3

f32 = mybir.dt.float32
bf16 = mybir.dt.bfloat16
AF = mybir.ActivationFunctionType

X_DT = bf16
W_DT = bf16
H_DT = bf16

# x chunk-group splits for the first and last block (sync ring): first
# block lands progressively during the cold-clock window, last block
# lands progressively so the tail tracks DMA supply
EDGE_SPLITS = [(0, 2), (2, 4), (4, 6), (6, 7)]
N_WARMUP = 2

_CACHE = {}


def _build(bc=BC):
    """Build the single-core Bass program (SPMD across 8 cores)."""
    nblk = bc // NBLK_COLS
    nc = bacc.Bacc()
    # xT is host-packed partition-major: [partition k, block, k-chunk c,
    # batch col] so any [:, b0:b1] slice is one fully contiguous DMA.
    xT = nc.declare_dram_parameter("xT", [KCH, nblk, NKC, NBLK_COLS], X_DT, isOutput=False)
    # w1 blob: host-prearranged [112, 7, 256] (chunk-major per partition)
    w1e = nc.declare_dram_parameter("w1b", [KCH, NKC, HID], W_DT, isOutput=False)
    # w2 blob: [128, 2, 10] (chunk-major per partition)
    w2 = nc.declare_dram_parameter("w2b", [128, 2, NCLS], W_DT, isOutput=False)
    # small-weights blob: [128, WB_W] fp32, see WB_* offsets
    wb = nc.declare_dram_parameter("wb", [128, WB_W], f32, isOutput=False)
    # output is logitsT [10, bc]; host transposes back
    out = nc.declare_dram_parameter("out", [NCLS, bc], f32, isOutput=True)

    with TileContext(nc) as tc:
        with (
            tc.tile_pool(name="weights", bufs=1) as wpool,
            tc.tile_pool(name="h_sb", bufs=4) as hpool,
            tc.tile_pool(name="h_ps", bufs=4, space="PSUM") as hps,
            tc.tile_pool(name="log_ps", bufs=2, space="PSUM") as logps,
        ):
            # ---- weight staging on the scalar HWDGE ring: smalls first,
            # then w1 chunk 0 (unblocks MM1), then the rest of w1 ----
            wb_sb = wpool.tile([128, WB_W], f32)
            nc.scalar.dma_start(out=wb_sb[:], in_=wb[:])
            w2_sb = wpool.tile([128, 2, NCLS], W_DT)
            nc.scalar.dma_start(out=w2_sb[:], in_=w2[:])
            w1_sb = wpool.tile([KCH, NKC, HID], W_DT)
            nc.scalar.dma_start(out=w1_sb[:, 0:1, :], in_=w1e[:, 0:1, :])
            nc.scalar.dma_start(out=w1_sb[:, 1:NKC, :], in_=w1e[:, 1:NKC, :])

            # ---- x staging: everything on the sync HWDGE ring in strict
            # consumption order (single queue = strict FIFO completion).
            # First and last block in chunk-pair groups; middle blocks as
            # one contiguous 800KB DMA each ----
            x_sb = wpool.tile([KCH, nblk, NKC, NBLK_COLS], X_DT)
            last_blk = nblk - 1
            for c0, c1 in EDGE_SPLITS:
                nc.sync.dma_start(out=x_sb[:, 0, c0:c1, :], in_=xT[:, 0, c0:c1, :])
            for b in range(1, last_blk):
                nc.sync.dma_start(out=x_sb[:, b : b + 1], in_=xT[:, b : b + 1])
            if last_blk > 0:
                for c0, c1 in EDGE_SPLITS:
                    nc.sync.dma_start(
                        out=x_sb[:, last_blk, c0:c1, :], in_=xT[:, last_blk, c0:c1, :]
                    )

            b1_sb = wb_sb[:, WB_B1:WB_B2]
            b2_sb = wb_sb[:NCLS, WB_B2:WB_W]
            # all blocks' logitsT accumulate here; per-block drain
            log_all = wpool.tile([NCLS, bc], f32)

            # tiny warm-up: start the HAM activity window while block-0
            # DMAs land (2 matmuls on a zeroed tile)
            warm_a = wpool.tile([KCH, 128], X_DT)
            warm_b = wpool.tile([KCH, NBLK_COLS], X_DT)
            nc.vector.memset(warm_a[:], 0.0)
            nc.vector.memset(warm_b[:], 0.0)
            warm_ps = hps.tile([128, NBLK_COLS], f32, tag="h_ps")
            for _ in range(N_WARMUP):
                nc.tensor.matmul(
                    warm_ps[:], warm_a[:], warm_b[:], start=True, stop=True,
                    skip_group_check=True,
                )

            # ---- main pipeline over 512-column blocks ----
            # MM2 for block n is emitted during block n+1's MM1 so the PE
            # never waits on the relu round-trip.
            pending = None  # (hs, b0) awaiting MM2

            def emit_mm2(hs, b0, last=False):
                log_ps = logps.tile([NCLS, NBLK_COLS], f32)
                for mc in range(2):
                    nc.tensor.matmul(
                        log_ps[:],
                        w2_sb[:, mc, :],
                        hs[mc][:],
                        start=(mc == 0),
                        stop=(mc == 1),
                    )
                nc.vector.tensor_scalar_add(
                    out=log_all[:, b0 : b0 + NBLK_COLS],
                    in0=log_ps[:],
                    scalar1=b2_sb[:, 0:1],
                )
                eng = nc.scalar if last else nc.gpsimd
                eng.dma_start(
                    out=out[:, b0 : b0 + NBLK_COLS],
                    in_=log_all[:, b0 : b0 + NBLK_COLS],
                )

            for blk in range(nblk - 1):
                b0 = blk * NBLK_COLS
                hs = []
                for mc in range(2):
                    h_ps = hps.tile([128, NBLK_COLS], f32)
                    for kc in range(NKC):
                        nc.tensor.matmul(
                            h_ps[:],
                            w1_sb[:, kc, mc * 128 : (mc + 1) * 128],
                            x_sb[:, blk, kc, :],
                            start=(kc == 0),
                            stop=(kc == NKC - 1),
                        )
                    h_sb = hpool.tile([128, NBLK_COLS], H_DT, tag="h")
                    nc.scalar.activation(
                        h_sb[:], h_ps[:], AF.Relu, bias=b1_sb[:, mc : mc + 1]
                    )
                    hs.append(h_sb)
                    if mc == 0 and pending is not None:
                        emit_mm2(*pending)
                        pending = None
                pending = (hs, b0)

            # last block: interleave mc0/mc1 per k-chunk so each chunk is
            # consumed (twice) as soon as its DMA lands, and both h halves
            # close ~2 matmuls after the final chunk arrives
            b0 = last_blk * NBLK_COLS
            h_ps2 = [
                hps.tile([128, NBLK_COLS], f32, name=f"h_ps_last{mc}", tag="h_ps")
                for mc in range(2)
            ]
            for kc in range(NKC):
                for mc in range(2):
                    nc.tensor.matmul(
                        h_ps2[mc][:],
                        w1_sb[:, kc, mc * 128 : (mc + 1) * 128],
                        x_sb[:, last_blk, kc, :],
                        start=(kc == 0),
                        stop=(kc == NKC - 1),
                    )
                if kc == 1 and pending is not None:
                    emit_mm2(*pending)
                    pending = None
            if pending is not None:  # nblk == 1
                emit_mm2(*pending)
                pending = None
            hs = []
            for mc in range(2):
                h_sb = hpool.tile([128, NBLK_COLS], H_DT, tag="h")
                nc.scalar.activation(
                    h_sb[:], h_ps2[mc][:], AF.Relu, bias=b1_sb[:, mc : mc + 1]
                )
                hs.append(h_sb)
            emit_mm2(hs, b0, last=True)

    nc.compile()
    return nc


def _fold_conv_into_w1(conv_w, W1):
    """W1eff[784, 256] such that x @ W1eff == conv(x) flattened @ W1."""
    conv_w = np.asarray(conv_w, dtype=np.float64)
    W1 = np.asarray(W1, dtype=np.float64)
    C = np.zeros((IMG, IMG, OUT_HW, OUT_HW), dtype=np.float64)
    oi = np.arange(OUT_HW)[:, None]
    oj = np.arange(OUT_HW)[None, :]
    for ki in range(KSZ):
        for kj in range(KSZ):
            C[oi + ki, oj + kj, oi, oj] = conv_w[ki, kj]
    W1eff = C.reshape(PIX, FEAT) @ W1
    return np.ascontiguousarray(W1eff, dtype=np.float32)


def _pack_weights(w1e, b1, W2, b2):
    # w1 blob [112, 7, 256]: chunk-major per partition (matches w1_sb)
    np_wdt = mybir.dt.np(W_DT)
    w1b = np.ascontiguousarray(
        w1e.reshape(NKC, KCH, HID).transpose(1, 0, 2).astype(np_wdt)
    )
    w2b = np.ascontiguousarray(
        W2.reshape(2, 128, NCLS).transpose(1, 0, 2).astype(np_wdt)
    )
    wb = np.zeros((128, WB_W), dtype=np.float32)
    wb[:, WB_B1:WB_B2] = b1.reshape(2, 128).T
    wb[:NCLS, WB_B2] = b2
    return w1b, w2b, wb


def kernel(x, conv_w, W1, b1, W2, b2, _bc=BC, _trace=False):
    x = np.asarray(x, dtype=np.float32)
    w1e = _fold_conv_into_w1(conv_w, W1)
    b1 = np.asarray(b1, dtype=np.float32)
    W2 = np.asarray(W2, dtype=np.float32)
    b2 = np.asarray(b2, dtype=np.float32)
    w1b, w2b, wb = _pack_weights(w1e, b1, W2, b2)

    n_cores = x.shape[0] // _bc
    if _bc not in _CACHE:
        _CACHE[_bc] = _build(_bc)
    nc = _CACHE[_bc]

    nblk = _bc // NBLK_COLS
    in_maps = [
        {
            # [bc, 784] -> [nblk, 512, 7, 112] -> [112, nblk, 7, 512]
            "xT": np.ascontiguousarray(
                x[c * _bc : (c + 1) * _bc]
                .reshape(nblk, NBLK_COLS, NKC, KCH)
                .transpose(3, 0, 2, 1)
                .astype(mybir.dt.np(X_DT))
            ),
            "w1b": w1b,
            "w2b": w2b,
            "wb": wb,
        }
        for c in range(n_cores)
    ]
    res = run_bass_kernel_spmd(
        nc, in_maps, core_ids=list(range(n_cores)), trace=_trace
    )
    # device layout logitsT [10, bc] -> [bc, 10]
    out = np.concatenate(
        [np.ascontiguousarray(res.results[c]["out"].T) for c in range(n_cores)],
        axis=0,
    )
    if _trace:
        return out, res
    return out


# revision 8
# speedup vs baseline: 1.1887x; 1.0299x over previous
"""Trainium2 Bass kernel for DigitConvolutionalModel.

Math: logits = relu(conv2d_valid(x.reshape(B,28,28), conv_w).reshape(B,676) @ W1 + b1) @ W2 + b2

Optimizations:
  1. The valid 3x3 conv is linear in x, so it folds into W1 on host:
     feat @ W1 == x @ (C @ W1) where C[784,676] scatters conv_w taps.
     The device then runs two dense matmuls per batch shard:
       h = relu(x @ W1eff + b1);  logits = h @ W2 + b2
  2. Sharding: batch 32768 split as 8 x 4096 across cores. The 784-pixel
     contraction is chunked 6x128 + 16 so the bulk of x moves in
     full-128-partition DMAs (measured ~20% faster per byte than
     112-partition transfers); the 16-pixel remainder (131KB) rides the
     gpsimd SWDGE ring. Host packs x partition-major ([128, nblk, 6,
     512] bf16) so every DMA is fully contiguous per partition.
  3. x, W1eff, h, W2 are bf16 on device (fp32 PSUM accumulation): halves
     DMA bytes AND streams the PE at 1 cycle/row. Measured end-to-end
     rel err ~3.6e-3 against the fp32 reference.
  4. DMA schedule (the kernel is PE-bound once supply is right): the
     sync HWDGE ring carries w1-main then all of x in consumption order
     (block 0 in 2-chunk pieces for progressive start, then one ~790KB
     DMA per block -- the measured single-ring sweet spot). Everything
     lands ~10us before the PE needs it; queues never cross-starve
     (heavily loaded rings starve light ones, so ordering is explicit).
  5. The PE starts real work ~8.5us (right after the framework preamble
     + first transfers) with 2 dummy warm-up matmuls to open the HAM
     clock-ramp window; the stream then runs gap-free so the clock
     stays at 2.4GHz. The last block interleaves its two output halves
     per k-chunk to shorten the relu->MM2->bias->out tail.

Device kernel (per core, per 512-column block):
  - MM1: hT[256,512] = W1eff.T @ xT, 6 k-chunks of 128 + 1 of 16
  - ACT: relu(hT + b1) PSUM->SBUF, output bf16
  - MM2 (pipelined one block behind): logitsT[10,512] over 2 chunks
  - DVE: + b2 (per-partition scalar add) PSUM->SBUF logitsT buffer
  - per-block DMA of logitsT slice; host transposes back to [B, 10]
"""
import ml_dtypes
import numpy as np

import concourse.bacc as bacc
import concourse.mybir as mybir
from concourse.tile import TileContext
from concourse.bass_utils import run_bass_kernel_spmd

B = 32768
IMG = 28
KSZ = 3
OUT_HW = IMG - KSZ + 1  # 26
FEAT = OUT_HW * OUT_HW  # 676
PIX = IMG * IMG  # 784
HID = 256
NCLS = 10
N_CORES = 8
BC = B // N_CORES  # 4096 rows per core
NBLK_COLS = 512  # batch columns per pipeline block (1 PSUM bank of fp32)
KCH = 128  # main contraction chunk: 784 = 6*128 + 16
NKC = 6
KREM = PIX - NKC * KCH  # 16
# wb blob layout (per partition, fp32 words): b1[2] | b2
WB_B1 = 0          # [128, 2]
WB_B2 = 2          # [128, 1] (only partitions 0..9 used)
WB_W = 3

f32 = mybir.dt.float32
bf16 = mybir.dt.bfloat16
AF = mybir.ActivationFunctionType

X_DT = bf16
W_DT = bf16
H_DT = bf16

N_WARMUP = 2

_CACHE = {}


def _build(bc=BC):
    """Build the single-core Bass program (SPMD across 8 cores)."""
    nblk = bc // NBLK_COLS
    nc = bacc.Bacc()
    # x main part, host-packed partition-major: [partition p, block,
    # k-chunk c, batch col]; pixel index = c*128 + p for c < 6
    xm = nc.declare_dram_parameter("xm", [KCH, nblk, NKC, NBLK_COLS], X_DT, isOutput=False)
    # x remainder: pixels 768..783 -> [16, nblk, 512]
    xr = nc.declare_dram_parameter("xr", [KREM, nblk, NBLK_COLS], X_DT, isOutput=False)
    # w1 main [128, 6, 256] (chunk-major per partition) + remainder [16, 256]
    w1m = nc.declare_dram_parameter("w1m", [KCH, NKC, HID], W_DT, isOutput=False)
    w1r = nc.declare_dram_parameter("w1r", [KREM, HID], W_DT, isOutput=False)
    # w2 blob: [128, 2, 10] (chunk-major per partition)
    w2 = nc.declare_dram_parameter("w2b", [128, 2, NCLS], W_DT, isOutput=False)
    # small-weights blob: [128, WB_W] fp32, see WB_* offsets
    wb = nc.declare_dram_parameter("wb", [128, WB_W], f32, isOutput=False)
    # output is logitsT [10, bc]; host transposes back
    out = nc.declare_dram_parameter("out", [NCLS, bc], f32, isOutput=True)

    with TileContext(nc) as tc:
        with (
            tc.tile_pool(name="weights", bufs=1) as wpool,
            tc.tile_pool(name="h_sb", bufs=4) as hpool,
            tc.tile_pool(name="h_ps", bufs=4, space="PSUM") as hps,
            tc.tile_pool(name="log_ps", bufs=2, space="PSUM") as logps,
        ):
            # ---- sync HWDGE ring, strict consumption order:
            # w1 chunk 0, w1 chunks 1-5, then all of x-main ----
            w1m_sb = wpool.tile([KCH, NKC, HID], W_DT)
            nc.sync.dma_start(out=w1m_sb[:, 0:1, :], in_=w1m[:, 0:1, :])
            nc.sync.dma_start(out=w1m_sb[:, 1:NKC, :], in_=w1m[:, 1:NKC, :])

            xm_sb = wpool.tile([KCH, nblk, NKC, NBLK_COLS], X_DT)
            last_blk = nblk - 1
            for c0, c1 in [(0, 2), (2, 4), (4, 6)]:
                nc.sync.dma_start(out=xm_sb[:, 0, c0:c1, :], in_=xm[:, 0, c0:c1, :])
            for b in range(1, nblk):
                nc.sync.dma_start(out=xm_sb[:, b : b + 1], in_=xm[:, b : b + 1])

            # ---- tiny weights on the scalar ring (needed later; safe
            # even if the loaded sync ring starves this one for a while)
            wb_sb = wpool.tile([128, WB_W], f32)
            nc.scalar.dma_start(out=wb_sb[:], in_=wb[:])
            w2_sb = wpool.tile([128, 2, NCLS], W_DT)
            nc.scalar.dma_start(out=w2_sb[:], in_=w2[:])
            w1r_sb = wpool.tile([KREM, HID], W_DT)
            nc.scalar.dma_start(out=w1r_sb[:], in_=w1r[:])

            # ---- x remainder on the gpsimd SWDGE ring, split so piece k
            # lands before block k's 7th k-chunk needs it ----
            xr_sb = wpool.tile([KREM, nblk, NBLK_COLS], X_DT)
            xr_splits = [(0, 1), (1, 3), (3, nblk)] if nblk > 3 else [(0, nblk)]
            for b0_, b1_ in xr_splits:
                nc.gpsimd.dma_start(out=xr_sb[:, b0_:b1_], in_=xr[:, b0_:b1_])

            b1_sb = wb_sb[:, WB_B1:WB_B2]
            b2_sb = wb_sb[:NCLS, WB_B2:WB_W]
            # all blocks' logitsT accumulate here; per-block drain
            log_all = wpool.tile([NCLS, bc], f32)

            # tiny warm-up: start the HAM activity window while block-0
            # DMAs land (2 matmuls on a zeroed tile)
            warm_a = wpool.tile([KCH, 128], X_DT)
            warm_b = wpool.tile([KCH, NBLK_COLS], X_DT)
            nc.vector.memset(warm_a[:], 0.0)
            nc.vector.memset(warm_b[:], 0.0)
            warm_ps = hps.tile([128, NBLK_COLS], f32, tag="h_ps")
            for _ in range(N_WARMUP):
                nc.tensor.matmul(
                    warm_ps[:], warm_a[:], warm_b[:], start=True, stop=True,
                    skip_group_check=True,
                )

            # ---- main pipeline over 512-column blocks ----
            # MM2 for block n is emitted during block n+1's MM1 so the PE
            # never waits on the relu round-trip.
            pending = None  # (hs, b0) awaiting MM2

            def emit_mm2(hs, b0, last=False):
                log_ps = logps.tile([NCLS, NBLK_COLS], f32)
                for mc in range(2):
                    nc.tensor.matmul(
                        log_ps[:],
                        w2_sb[:, mc, :],
                        hs[mc][:],
                        start=(mc == 0),
                        stop=(mc == 1),
                    )
                nc.vector.tensor_scalar_add(
                    out=log_all[:, b0 : b0 + NBLK_COLS],
                    in0=log_ps[:],
                    scalar1=b2_sb[:, 0:1],
                )
                eng = nc.scalar if last else nc.gpsimd
                eng.dma_start(
                    out=out[:, b0 : b0 + NBLK_COLS],
                    in_=log_all[:, b0 : b0 + NBLK_COLS],
                )

            def mm1(h_ps, blk, mc, kc):
                if kc < NKC:
                    nc.tensor.matmul(
                        h_ps[:],
                        w1m_sb[:, kc, mc * 128 : (mc + 1) * 128],
                        xm_sb[:, blk, kc, :],
                        start=(kc == 0),
                        stop=False,
                    )
                else:
                    nc.tensor.matmul(
                        h_ps[:],
                        w1r_sb[:, mc * 128 : (mc + 1) * 128],
                        xr_sb[:, blk, :],
                        start=False,
                        stop=True,
                    )

            for blk in range(nblk - 1):
                b0 = blk * NBLK_COLS
                hs = []
                for mc in range(2):
                    h_ps = hps.tile([128, NBLK_COLS], f32)
                    for kc in range(NKC + 1):
                        mm1(h_ps, blk, mc, kc)
                    h_sb = hpool.tile([128, NBLK_COLS], H_DT, tag="h")
                    nc.scalar.activation(
                        h_sb[:], h_ps[:], AF.Relu, bias=b1_sb[:, mc : mc + 1]
                    )
                    hs.append(h_sb)
                    if mc == 0 and pending is not None:
                        emit_mm2(*pending)
                        pending = None
                pending = (hs, b0)

            # last block: interleave mc0/mc1 per k-chunk so both h halves
            # close right after the final chunk, shortening the tail
            b0 = last_blk * NBLK_COLS
            h_ps2 = [
                hps.tile([128, NBLK_COLS], f32, name=f"h_ps_last{mc}", tag="h_ps")
                for mc in range(2)
            ]
            for kc in range(NKC + 1):
                for mc in range(2):
                    mm1(h_ps2[mc], last_blk, mc, kc)
                if kc == 1 and pending is not None:
                    emit_mm2(*pending)
                    pending = None
            if pending is not None:  # nblk == 1
                emit_mm2(*pending)
                pending = None
            hs = []
            for mc in range(2):
                h_sb = hpool.tile([128, NBLK_COLS], H_DT, tag="h", name=f"h_last{mc}")
                nc.scalar.activation(
                    h_sb[:], h_ps2[mc][:], AF.Relu, bias=b1_sb[:, mc : mc + 1]
                )
                hs.append(h_sb)
            emit_mm2(hs, b0, last=True)

    nc.compile()
    return nc


def _fold_conv_into_w1(conv_w, W1):
    """W1eff[784, 256] such that x @ W1eff == conv(x) flattened @ W1."""
    conv_w = np.asarray(conv_w, dtype=np.float64)
    W1 = np.asarray(W1, dtype=np.float64)
    C = np.zeros((IMG, IMG, OUT_HW, OUT_HW), dtype=np.float64)
    oi = np.arange(OUT_HW)[:, None]
    oj = np.arange(OUT_HW)[None, :]
    for ki in range(KSZ):
        for kj in range(KSZ):
            C[oi + ki, oj + kj, oi, oj] = conv_w[ki, kj]
    W1eff = C.reshape(PIX, FEAT) @ W1
    return np.ascontiguousarray(W1eff, dtype=np.float32)


def _pack_weights(w1e, b1, W2, b2):
    np_wdt = mybir.dt.np(W_DT)
    w1m = np.ascontiguousarray(
        w1e[: NKC * KCH].reshape(NKC, KCH, HID).transpose(1, 0, 2).astype(np_wdt)
    )
    w1r = np.ascontiguousarray(w1e[NKC * KCH :].astype(np_wdt))
    w2b = np.ascontiguousarray(
        W2.reshape(2, 128, NCLS).transpose(1, 0, 2).astype(np_wdt)
    )
    wb = np.zeros((128, WB_W), dtype=np.float32)
    wb[:, WB_B1:WB_B2] = b1.reshape(2, 128).T
    wb[:NCLS, WB_B2] = b2
    return w1m, w1r, w2b, wb


def kernel(x, conv_w, W1, b1, W2, b2, _bc=BC, _trace=False):
    x = np.asarray(x, dtype=np.float32)
    w1e = _fold_conv_into_w1(conv_w, W1)
    b1 = np.asarray(b1, dtype=np.float32)
    W2 = np.asarray(W2, dtype=np.float32)
    b2 = np.asarray(b2, dtype=np.float32)
    w1m, w1r, w2b, wb = _pack_weights(w1e, b1, W2, b2)

    n_cores = x.shape[0] // _bc
    if _bc not in _CACHE:
        _CACHE[_bc] = _build(_bc)
    nc = _CACHE[_bc]

    nblk = _bc // NBLK_COLS
    np_xdt = mybir.dt.np(X_DT)
    in_maps = []
    for c in range(n_cores):
        xc = x[c * _bc : (c + 1) * _bc]
        in_maps.append(
            {
                # [bc, 768] -> [nblk, 512, 6, 128] -> [128, nblk, 6, 512]
                "xm": np.ascontiguousarray(
                    xc[:, : NKC * KCH]
                    .reshape(nblk, NBLK_COLS, NKC, KCH)
                    .transpose(3, 0, 2, 1)
                    .astype(np_xdt)
                ),
                # [bc, 16] -> [16, nblk, 512]
                "xr": np.ascontiguousarray(
                    xc[:, NKC * KCH :]
                    .reshape(nblk, NBLK_COLS, KREM)
                    .transpose(2, 0, 1)
                    .astype(np_xdt)
                ),
                "w1m": w1m,
                "w1r": w1r,
                "w2b": w2b,
                "wb": wb,
            }
        )
    res = run_bass_kernel_spmd(
        nc, in_maps, core_ids=list(range(n_cores)), trace=_trace
    )
    # device layout logitsT [10, bc] -> [bc, 10]
    out = np.concatenate(
        [np.ascontiguousarray(res.results[c]["out"].T) for c in range(n_cores)],
        axis=0,
    )
    if _trace:
        return out, res
    return out
